# revision 1
# baseline (speedup 1.0000x reference)
"""DCRNN (2-layer DCGRU encoder/decoder, K=2 Chebyshev) Trainium2 kernel.

Sharding: pure data-parallel over batch B=128 -> 16 samples per core x 8 cores.

Layouts (per core, BL=16 samples, N=64 nodes, NT=BL*N=1024):
  feature-major state tiles: [feat_partition, 64*b + n]
  samples paired (2 per 128-partition group) for block-diagonal support matmuls.

Per DCGRU cell (layer l, feature dim F = Dx + 64):
  gate = sigmoid(cat0 @ Wg0' + (S@cat0) @ Wg1 + (S2@cat0) @ Wg2' + bg)
  with Wg0' = Wg0 - Wg2, Wg2' = 2*Wg2  (since cat2 = 2*S2@cat0 - cat0)
  computed feature-major via: per-pair PE transpose of cat0 (fm->nm), one
  matmul per pair against [ST|S2T] block-diag tiles (fm diffusion outputs),
  then weight matmuls with W stationary streaming all 16 samples.
"""

import contextlib

import numpy as np
import ml_dtypes

import concourse.bass as bass
import concourse.mybir as mybir
import concourse.tile as tile
from concourse import bacc
from concourse.bass_utils import run_bass_kernel_spmd
from concourse.masks import make_identity

F32 = mybir.dt.float32
BF16 = mybir.dt.bfloat16
AF = mybir.ActivationFunctionType

B, TIN, TOUT, N, H = 128, 64, 32, 64, 64
NCORES = 8
BL = B // NCORES          # 16 samples per core
PAIRS = BL // 2           # 8
NT = BL * N               # 1024 node-columns per core
F0, F1 = 1 + H, H + H     # 65, 128

_CACHE = {}
last_exec_wall_ns = None  # wall time of the device dispatch in the last call


# ----------------------------------------------------------------------------
# device kernel builder
# ----------------------------------------------------------------------------

def _emit_cell(nc, pools, tiles, lay, sbuf_sts, dbg=""):
    """Emit one DCGRU cell. lay: dict with F, Dx, state, cand, cc, wg, wc,
    bg, bc, h_dests (list of (tile, row0) to write h' into)."""
    F, Dx = lay["F"], lay["Dx"]
    state, cand, cc = lay["state"], lay["cand"], lay["cc"]
    wg, wc, bgt, bct = lay["wg"], lay["wc"], lay["bg"], lay["bc"]
    ident = tiles["ident"]
    r_t, u_t = lay["r"], lay["u"]
    c_t, d_t, e_t = lay["c"], lay["d"], lay["e"]
    pT, pD, pG, pC = pools["pT"], pools["pD"], pools["pG"], pools["pC"]
    nm_pool = pools["nm"]

    # --- gate path: per-pair transpose + diffusion ---
    for p in range(PAIRS):
        ps_t1 = pT.tile([128, 128], BF16, tag="pT")
        nc.tensor.transpose(ps_t1[:, :F], state[:, p * 128:(p + 1) * 128],
                            ident[:F, :F])
        cat0nm = nm_pool.tile([128, 128], BF16, tag="nm")
        nc.vector.tensor_copy(cat0nm[:, :F], ps_t1[:, :F])
        ps_d1 = pD.tile([128, 256], F32, tag="pD")
        nc.tensor.matmul(ps_d1[:F, :], cat0nm[:, :F],
                         sbuf_sts[:, p * 256:(p + 1) * 256],
                         start=True, stop=True)
        # alternate copy engine: ACT copies are ~2x slower than DVE, so
        # split the 8 per-pair copies between the two engines
        if p % 2 == 0:
            nc.vector.tensor_copy(cc[:F, p * 256:(p + 1) * 256], ps_d1[:F, :])
        else:
            nc.scalar.copy(cc[:F, p * 256:(p + 1) * 256], ps_d1[:F, :])

    # --- gate weight matmuls (W stationary, all samples streamed) ---
    cc_r = cc[:].rearrange("f (p c) -> f p c", c=256)
    for h in range(2):
        ps_g = pG.tile([128, 512], F32, tag="pG")
        nc.tensor.matmul(ps_g[:], wg[:, 0:128], state[:, h * 512:(h + 1) * 512],
                         start=True, stop=False)
        nc.tensor.matmul(ps_g[:], wg[:, 128:256],
                         cc_r[:F, 4 * h:4 * h + 4, 0:128],
                         start=False, stop=False)
        nc.tensor.matmul(ps_g[:], wg[:, 256:384],
                         cc_r[:F, 4 * h:4 * h + 4, 128:256],
                         start=False, stop=True)
        nc.scalar.activation(r_t[:, h * 512:(h + 1) * 512], ps_g[0:64, :],
                             AF.Sigmoid, bias=bgt[0:64, 0:1])
        nc.scalar.activation(u_t[:, h * 512:(h + 1) * 512], ps_g[64:128, :],
                             AF.Sigmoid, bias=bgt[64:128, 0:1])

    # --- candidate path ---
    # rh = r * h  written into cand rows [0, 64)
    nc.vector.tensor_mul(cand[0:64, :], r_t[:, :], state[0:64, :])
    for p in range(PAIRS):
        ps_t2 = pT.tile([128, 128], BF16, tag="pT")
        nc.tensor.transpose(ps_t2[:, :64], cand[0:64, p * 128:(p + 1) * 128],
                            ident[0:64, 0:64])
        rhnm = nm_pool.tile([128, 128], BF16, tag="nm")
        if p % 2 == 0:
            nc.vector.tensor_copy(rhnm[:, :64], ps_t2[:, :64])
        else:
            nc.scalar.copy(rhnm[:, :64], ps_t2[:, :64])
        ps_d2 = pD.tile([128, 256], F32, tag="pD")
        nc.tensor.matmul(ps_d2[:64, :], rhnm[:, :64],
                         sbuf_sts[:, p * 256:(p + 1) * 256],
                         start=True, stop=True)
        if p % 2 == 0:
            nc.vector.tensor_copy(cc[0:64, p * 256:(p + 1) * 256],
                                  ps_d2[:64, :])
        else:
            nc.scalar.copy(cc[0:64, p * 256:(p + 1) * 256], ps_d2[:64, :])

    for h in range(2):
        ps_c = pC.tile([64, 512], F32, tag="pC")
        nc.tensor.matmul(ps_c[:], wc[:, 0:64], cand[:, h * 512:(h + 1) * 512],
                         start=True, stop=False)
        nc.tensor.matmul(ps_c[:], wc[:, 64:128],
                         cc_r[:F, 4 * h:4 * h + 4, 0:128],
                         start=False, stop=False)
        nc.tensor.matmul(ps_c[:], wc[:, 128:192],
                         cc_r[:F, 4 * h:4 * h + 4, 128:256],
                         start=False, stop=True)
        nc.scalar.activation(c_t[:, h * 512:(h + 1) * 512], ps_c[:],
                             AF.Tanh, bias=bct[:, 0:1])

    # --- GRU update: h' = c + u * (h - c) ---
    nc.vector.tensor_sub(d_t[:], state[0:64, :], c_t[:])
    nc.vector.tensor_mul(e_t[:], u_t[:, :], d_t[:])
    dest0, extra = lay["h_dest"], lay["h_copies"]
    nc.vector.tensor_add(dest0, c_t[:], e_t[:])
    for dst in extra:
        nc.gpsimd.tensor_copy(dst, dest0)


def _build(tin, tout):
    nc = bacc.Bacc("TRN2", target_bir_lowering=False, debug=False)

    # ---- DRAM parameters ----
    sts2 = nc.declare_dram_parameter("sts2", [tin, 4, 64, PAIRS, 64], BF16,
                                     isOutput=False)
    xenc = nc.declare_dram_parameter("xenc", [tin, NT], BF16, isOutput=False)
    go = nc.declare_dram_parameter("go", [1, NT], BF16, isOutput=False)
    wgs, wcs, bgs, bcs = {}, {}, {}, {}
    for m, F in [("e0", F0), ("e1", F1), ("d0", F0), ("d1", F1)]:
        wgs[m] = nc.declare_dram_parameter(f"wg_{m}", [F, 384], BF16,
                                           isOutput=False)
        wcs[m] = nc.declare_dram_parameter(f"wc_{m}", [F, 192], BF16,
                                           isOutput=False)
        bgs[m] = nc.declare_dram_parameter(f"bg_{m}", [128, 1], F32,
                                           isOutput=False)
        bcs[m] = nc.declare_dram_parameter(f"bc_{m}", [64, 1], F32,
                                           isOutput=False)
    pw = nc.declare_dram_parameter("pw", [128, 1], BF16, isOutput=False)
    pb = nc.declare_dram_parameter("pb", [1, 1], BF16, isOutput=False)
    y = nc.declare_dram_parameter("y", [tout, NT], F32, isOutput=True)

    with tile.TileContext(nc) as tc:
        import contextlib
        with contextlib.ExitStack() as ctx:
            persist = ctx.enter_context(tc.tile_pool(name="persist", bufs=1))
            nm_pool = ctx.enter_context(tc.tile_pool(name="nm", bufs=8))
            pT = ctx.enter_context(tc.tile_pool(name="pT", bufs=2, space="PSUM"))
            pD = ctx.enter_context(tc.tile_pool(name="pD", bufs=2, space="PSUM"))
            pG = ctx.enter_context(tc.tile_pool(name="pG", bufs=2, space="PSUM"))
            pC = ctx.enter_context(tc.tile_pool(name="pC", bufs=2, space="PSUM"))
            pools = {"pT": pT, "pD": pD, "pG": pG, "pC": pC, "nm": nm_pool}

            ident = persist.tile([128, 128], BF16)
            make_identity(nc, ident[:])

            stss = [persist.tile([128, PAIRS * 256], BF16, name=f"stss{i}")
                    for i in range(2)]
            for s in stss:
                nc.gpsimd.memset(s[:], 0.0)

            st0 = persist.tile([F0, NT], BF16, name="st0")
            st1 = persist.tile([128, NT], BF16, name="st1")
            cnd0 = persist.tile([F0, NT], BF16, name="cnd0")
            cnd1 = persist.tile([128, NT], BF16, name="cnd1")
            cc0 = persist.tile([F0, PAIRS * 256], BF16, name="cc0")
            cc1 = persist.tile([128, PAIRS * 256], BF16, name="cc1")
            lt = {}
            for li in (0, 1):
                lt[li] = dict(
                    r=persist.tile([64, NT], BF16, name=f"r{li}"),
                    u=persist.tile([64, NT], BF16, name=f"u{li}"),
                    c=persist.tile([64, NT], BF16, name=f"c{li}"),
                    d=persist.tile([64, NT], BF16, name=f"d{li}"),
                    e=persist.tile([64, NT], BF16, name=f"e{li}"),
                )
            ones = persist.tile([1, NT], BF16, name="ones")
            nc.gpsimd.memset(ones[:], 1.0)
            ystage = persist.tile([1, NT], F32, name="ystage")

            nc.gpsimd.memset(st0[0:64, :], 0.0)
            nc.gpsimd.memset(st1[:, :], 0.0)

            wgt, wct, bgt, bct = {}, {}, {}, {}
            for m, F in [("e0", F0), ("e1", F1), ("d0", F0), ("d1", F1)]:
                wgt[m] = persist.tile([F, 384], BF16, name=f"wgt{m}")
                nc.sync.dma_start(wgt[m][:], wgs[m][:])
                wct[m] = persist.tile([F, 192], BF16, name=f"wct{m}")
                nc.sync.dma_start(wct[m][:], wcs[m][:])
                bgt[m] = persist.tile([128, 1], F32, name=f"bgt{m}")
                nc.sync.dma_start(bgt[m][:], bgs[m][:])
                bct[m] = persist.tile([64, 1], F32, name=f"bct{m}")
                nc.sync.dma_start(bct[m][:], bcs[m][:])
            pwt = persist.tile([128, 1], BF16, name="pwt")
            nc.sync.dma_start(pwt[:], pw[:])
            pbt = persist.tile([1, 1], BF16, name="pbt")
            nc.sync.dma_start(pbt[:], pb[:])

            tiles = {"ident": ident}

            # Row conventions (all h at base 0, x at the bottom):
            #   st0 [h0 (0:64), x (64:65)]    cnd0 [rh0 (0:64), x (64:65)]
            #   st1 [h1 (0:64), x=h0' (64:128)]  cnd1 [rh1 (0:64), x (64:128)]
            #   cc* rows [h-diff (0:64), x-diff (64:F)]
            # All weight matrices are row-permuted host-side to match.
            def lay0(m):
                return dict(F=F0, Dx=1, state=st0, cand=cnd0, cc=cc0,
                            wg=wgt[m], wc=wct[m], bg=bgt[m], bc=bct[m],
                            h_dest=st0[0:64, :],
                            h_copies=[st1[64:128, :], cnd1[64:128, :]],
                            **lt[0])

            def lay1(m):
                return dict(F=F1, Dx=64, state=st1, cand=cnd1, cc=cc1,
                            wg=wgt[m], wc=wct[m], bg=bgt[m], bc=bct[m],
                            h_dest=st1[0:64, :], h_copies=[], **lt[1])

            # ---------------- encoder ----------------
            for t in range(tin):
                sb = stss[t % 2]
                for q, (r0, c0) in enumerate([(0, 0), (64, 64), (0, 128),
                                              (64, 192)]):
                    dst = sb[r0:r0 + 64, :].rearrange("r (p c) -> r p c", c=256)
                    nc.sync.dma_start(dst[:, :, c0:c0 + 64], sts2[t, q])
                nc.sync.dma_start(st0[64:65, :], xenc[t:t + 1, :])
                nc.sync.dma_start(cnd0[64:65, :], xenc[t:t + 1, :])
                _emit_cell(nc, pools, tiles, lay0("e0"), sb)
                _emit_cell(nc, pools, tiles, lay1("e1"), sb)

            # ---------------- decoder ----------------
            sb = stss[(tin - 1) % 2]
            nc.sync.dma_start(st0[64:65, :], go[:])
            nc.sync.dma_start(cnd0[64:65, :], go[:])
            for t in range(tout):
                _emit_cell(nc, pools, tiles, lay0("d0"), sb)
                _emit_cell(nc, pools, tiles, lay1("d1"), sb)
                # projection: y_t = h1' @ pw + pb   (feature-major: [1, NT])
                for h in range(2):
                    ps_p = pC.tile([64, 512], F32, tag="pC")
                    nc.tensor.matmul(ps_p[0:1, :], pwt[:, :],
                                     st1[:, h * 512:(h + 1) * 512],
                                     start=True, stop=False)
                    nc.tensor.matmul(ps_p[0:1, :], pbt[:, :],
                                     ones[:, h * 512:(h + 1) * 512],
                                     start=False, stop=True)
                    hs = slice(h * 512, (h + 1) * 512)
                    # next-step x feedback is the decoder critical path:
                    # put the two halves on different engines so they run
                    # concurrently, and demote the y staging (not on the
                    # recurrence path) behind it
                    if t < tout - 1:
                        if h == 0:
                            nc.scalar.copy(st0[64:65, hs], ps_p[0:1, :])
                        else:
                            nc.vector.tensor_copy(st0[64:65, hs],
                                                  ps_p[0:1, :])
                    if h == 0:
                        nc.vector.tensor_copy(ystage[0:1, hs], ps_p[0:1, :])
                    else:
                        nc.scalar.copy(ystage[0:1, hs], ps_p[0:1, :])
                    nc.sync.dma_start(y[t:t + 1, hs], ystage[0:1, hs])
                if t < tout - 1:
                    # off the critical path (first read is at candW time)
                    nc.gpsimd.tensor_copy(cnd0[64:65, :], st0[64:65, :])

    nc.compile()
    return nc


# ----------------------------------------------------------------------------
# host side
# ----------------------------------------------------------------------------

def _prep_weights(Wg, bg, Wc, bc, F):
    """Split [3F, O] chebyshev-stacked weights, merge cat2 into cat0/s2 terms.

    Reference feature order within each Chebyshev block is [x (Dx), h (64)];
    on-chip tiles hold [h (0:64), x (64:F)], so every block's rows are
    permuted to [Dx:F, 0:Dx].
    """
    Dx = F - 64
    perm = list(range(Dx, F)) + list(range(Dx))
    Wg = np.asarray(Wg, np.float32)
    Wc = np.asarray(Wc, np.float32)
    w0, w1, w2 = Wg[0:F][perm], Wg[F:2 * F][perm], Wg[2 * F:3 * F][perm]
    wg = np.concatenate([w0 - w2, w1, 2.0 * w2], axis=1)  # [F, 384]
    c0, c1, c2 = Wc[0:F][perm], Wc[F:2 * F][perm], Wc[2 * F:3 * F][perm]
    wc = np.concatenate([c0 - c2, c1, 2.0 * c2], axis=1)  # [F, 192]
    return (wg.astype(ml_dtypes.bfloat16), wc.astype(ml_dtypes.bfloat16),
            np.asarray(bg, np.float32).reshape(-1, 1),
            np.asarray(bc, np.float32).reshape(-1, 1))


def kernel(encoder_inputs, decoder_inputs, supports,
           enc0_Wg, enc0_bg, enc0_Wc, enc0_bc,
           enc1_Wg, enc1_bg, enc1_Wc, enc1_bc,
           dec0_Wg, dec0_bg, dec0_Wc, dec0_bc,
           dec1_Wg, dec1_bg, dec1_Wc, dec1_bc,
           proj_W, proj_b):
    encoder_inputs = np.asarray(encoder_inputs, np.float32)
    decoder_inputs = np.asarray(decoder_inputs, np.float32)
    supports = np.asarray(supports, np.float32)
    Bv, tin, Nv, _ = encoder_inputs.shape
    tout = decoder_inputs.shape[1]

    key = (tin, tout)
    if key not in _CACHE:
        _CACHE[key] = _build(tin, tout)
    nc = _CACHE[key]

    # ST / S2T, block-diag pair quadrant layout  [T, 4, 64, PAIRS, 64]
    st = np.transpose(supports, (0, 1, 3, 2))                  # (B,T,N,N) S^T
    s2t = np.matmul(st, st)          # (S@S)^T = S^T @ S^T  (BLAS batched)
    st = st.astype(ml_dtypes.bfloat16)
    s2t = s2t.astype(ml_dtypes.bfloat16)

    wg_e0, wc_e0, bg_e0, bc_e0 = _prep_weights(enc0_Wg, enc0_bg, enc0_Wc,
                                               enc0_bc, F0)
    wg_e1, wc_e1, bg_e1, bc_e1 = _prep_weights(enc1_Wg, enc1_bg, enc1_Wc,
                                               enc1_bc, F1)
    wg_d0, wc_d0, bg_d0, bc_d0 = _prep_weights(dec0_Wg, dec0_bg, dec0_Wc,
                                               dec0_bc, F0)
    wg_d1, wc_d1, bg_d1, bc_d1 = _prep_weights(dec1_Wg, dec1_bg, dec1_Wc,
                                               dec1_bc, F1)
    pw_h = np.zeros((128, 1), np.float32)
    pw_h[0:64] = np.asarray(proj_W, np.float32).reshape(64, 1)
    pw_h = pw_h.astype(ml_dtypes.bfloat16)
    pb_h = np.asarray(proj_b, np.float32).reshape(1, 1).astype(
        ml_dtypes.bfloat16)

    in_maps = []
    for c in range(NCORES):
        bs = slice(c * BL, (c + 1) * BL)
        st_c = st[bs]        # (BL, T, 64, 64)
        s2t_c = s2t[bs]
        sts2 = np.empty((tin, 4, 64, PAIRS, 64), ml_dtypes.bfloat16)
        sts2[:, 0] = np.transpose(st_c[0::2], (1, 2, 0, 3))
        sts2[:, 1] = np.transpose(st_c[1::2], (1, 2, 0, 3))
        sts2[:, 2] = np.transpose(s2t_c[0::2], (1, 2, 0, 3))
        sts2[:, 3] = np.transpose(s2t_c[1::2], (1, 2, 0, 3))
        xe = np.transpose(encoder_inputs[bs, :, :, 0], (1, 0, 2)).reshape(
            tin, NT).astype(ml_dtypes.bfloat16)
        go_h = decoder_inputs[bs, 0, :, 0].reshape(1, NT).astype(
            ml_dtypes.bfloat16)
        in_maps.append({
            "sts2": sts2, "xenc": xe, "go": go_h,
            "wg_e0": wg_e0, "wc_e0": wc_e0, "bg_e0": bg_e0, "bc_e0": bc_e0,
            "wg_e1": wg_e1, "wc_e1": wc_e1, "bg_e1": bg_e1, "bc_e1": bc_e1,
            "wg_d0": wg_d0, "wc_d0": wc_d0, "bg_d0": bg_d0, "bc_d0": bc_d0,
            "wg_d1": wg_d1, "wc_d1": wc_d1, "bg_d1": bg_d1, "bc_d1": bc_d1,
            "pw": pw_h, "pb": pb_h,
        })

    global last_exec_wall_ns
    import time as _time
    _t0 = _time.time()
    res = run_bass_kernel_spmd(nc, in_maps, list(range(NCORES)), trace=False)
    last_exec_wall_ns = int((_time.time() - _t0) * 1e9)

    out = np.empty((Bv, tout, Nv, 1), np.float32)
    for c in range(NCORES):
        yc = res.results[c]["y"].reshape(tout, BL, Nv)
        out[c * BL:(c + 1) * BL, :, :, 0] = np.transpose(yc, (1, 0, 2))
    return out



# revision 5
# speedup vs baseline: 8.2351x; 8.2351x over previous
"""DCRNN (2-layer DCGRU encoder/decoder, K=2 Chebyshev) Trainium2 kernel.

Sharding: pure data-parallel over batch B=128 -> 16 samples per core x 8 cores.

Layouts (per core, BL=16 samples, N=64 nodes, NT=BL*N=1024):
  feature-major state tiles: [feat_partition, 64*b + n]
  samples paired (2 per 128-partition group) for block-diagonal support matmuls.

Host->device traffic is the bottleneck (axon tunnel ~100MB/s), so supports are
shipped once per step as uint8-quantized S^T only ([tin,128,512] contiguous);
the device converts u8->bf16 into the block-diag pair layout and computes
S2^T = S^T @ S^T itself (block-diag squared stays block-diag).  The
quantization scale s is folded into the gate/candidate weights host-side
(diffusion terms are linear in S): blocks [W0 - W2, s*W1, 2*s^2*W2].

Per DCGRU cell (layer l, feature dim F = Dx + 64):
  gate = sigmoid(cat0 @ Wg0' + (Si@cat0) @ (s Wg1) + (Si2@cat0) @ (2 s^2 Wg2) + bg)
  computed feature-major via: per-pair PE transpose of cat0 (fm->nm), one
  matmul per pair against [SiT|Si2T] block-diag tiles (fm diffusion outputs),
  then weight matmuls with W stationary streaming all 16 samples.

Execution path: the jitted shard_map(bass_exec) callable is cached per build —
re-creating it per call (as run_bass_kernel_spmd does under axon) re-lowers
and re-loads the NEFF, costing seconds per call.
"""

import numpy as np
import ml_dtypes

import concourse.bass as bass
import concourse.mybir as mybir
import concourse.tile as tile
from concourse import bacc
from concourse import bass2jax as _b2j
from concourse.masks import make_identity

F32 = mybir.dt.float32
BF16 = mybir.dt.bfloat16
U8 = mybir.dt.uint8
AF = mybir.ActivationFunctionType

B, TIN, TOUT, N, H = 128, 64, 32, 64, 64
NCORES = 8
BL = B // NCORES          # 16 samples per core
PAIRS = BL // 2           # 8
NT = BL * N               # 1024 node-columns per core
F0, F1 = 1 + H, H + H     # 65, 128

_CACHE = {}
last_exec_wall_ns = None  # wall time of the device dispatch in the last call


# ----------------------------------------------------------------------------
# device kernel builder
# ----------------------------------------------------------------------------

def _emit_cell(nc, pools, tiles, lay, sbuf_sts, dbg=""):
    """Emit one DCGRU cell. lay: dict with F, Dx, state, cand, cc, wg, wc,
    bg, bc, h_dests (list of (tile, row0) to write h' into)."""
    F, Dx = lay["F"], lay["Dx"]
    state, cand, cc = lay["state"], lay["cand"], lay["cc"]
    wg, wc, bgt, bct = lay["wg"], lay["wc"], lay["bg"], lay["bc"]
    ident = tiles["ident"]
    r_t, u_t = lay["r"], lay["u"]
    c_t, d_t, e_t = lay["c"], lay["d"], lay["e"]
    pT, pD, pG, pC = pools["pT"], pools["pD"], pools["pG"], pools["pC"]
    nm_pool = pools["nm"]

    # --- gate path: per-pair transpose + diffusion ---
    for p in range(PAIRS):
        ps_t1 = pT.tile([128, 128], BF16, tag="pT")
        nc.tensor.transpose(ps_t1[:, :F], state[:, p * 128:(p + 1) * 128],
                            ident[:F, :F])
        cat0nm = nm_pool.tile([128, 128], BF16, tag="nm")
        nc.vector.tensor_copy(cat0nm[:, :F], ps_t1[:, :F])
        ps_d1 = pD.tile([128, 256], F32, tag="pD")
        nc.tensor.matmul(ps_d1[:F, :], cat0nm[:, :F],
                         sbuf_sts[:, p * 256:(p + 1) * 256],
                         start=True, stop=True)
        # alternate copy engine: ACT copies are ~2x slower than DVE, so
        # split the 8 per-pair copies between the two engines
        if p % 2 == 0:
            nc.vector.tensor_copy(cc[:F, p * 256:(p + 1) * 256], ps_d1[:F, :])
        else:
            nc.scalar.copy(cc[:F, p * 256:(p + 1) * 256], ps_d1[:F, :])

    # --- gate weight matmuls (W stationary, all samples streamed) ---
    cc_r = cc[:].rearrange("f (p c) -> f p c", c=256)
    for h in range(2):
        ps_g = pG.tile([128, 512], F32, tag="pG")
        nc.tensor.matmul(ps_g[:], wg[:, 0:128], state[:, h * 512:(h + 1) * 512],
                         start=True, stop=False)
        nc.tensor.matmul(ps_g[:], wg[:, 128:256],
                         cc_r[:F, 4 * h:4 * h + 4, 0:128],
                         start=False, stop=False)
        nc.tensor.matmul(ps_g[:], wg[:, 256:384],
                         cc_r[:F, 4 * h:4 * h + 4, 128:256],
                         start=False, stop=True)
        nc.scalar.activation(r_t[:, h * 512:(h + 1) * 512], ps_g[0:64, :],
                             AF.Sigmoid, bias=bgt[0:64, 0:1])
        nc.scalar.activation(u_t[:, h * 512:(h + 1) * 512], ps_g[64:128, :],
                             AF.Sigmoid, bias=bgt[64:128, 0:1])

    # --- candidate path ---
    # rh = r * h  written into cand rows [0, 64)
    nc.vector.tensor_mul(cand[0:64, :], r_t[:, :], state[0:64, :])
    for p in range(PAIRS):
        ps_t2 = pT.tile([128, 128], BF16, tag="pT")
        nc.tensor.transpose(ps_t2[:, :64], cand[0:64, p * 128:(p + 1) * 128],
                            ident[0:64, 0:64])
        rhnm = nm_pool.tile([128, 128], BF16, tag="nm")
        if p % 2 == 0:
            nc.vector.tensor_copy(rhnm[:, :64], ps_t2[:, :64])
        else:
            nc.scalar.copy(rhnm[:, :64], ps_t2[:, :64])
        ps_d2 = pD.tile([128, 256], F32, tag="pD")
        nc.tensor.matmul(ps_d2[:64, :], rhnm[:, :64],
                         sbuf_sts[:, p * 256:(p + 1) * 256],
                         start=True, stop=True)
        if p % 2 == 0:
            nc.vector.tensor_copy(cc[0:64, p * 256:(p + 1) * 256],
                                  ps_d2[:64, :])
        else:
            nc.scalar.copy(cc[0:64, p * 256:(p + 1) * 256], ps_d2[:64, :])

    for h in range(2):
        ps_c = pC.tile([64, 512], F32, tag="pC")
        nc.tensor.matmul(ps_c[:], wc[:, 0:64], cand[:, h * 512:(h + 1) * 512],
                         start=True, stop=False)
        nc.tensor.matmul(ps_c[:], wc[:, 64:128],
                         cc_r[:F, 4 * h:4 * h + 4, 0:128],
                         start=False, stop=False)
        nc.tensor.matmul(ps_c[:], wc[:, 128:192],
                         cc_r[:F, 4 * h:4 * h + 4, 128:256],
                         start=False, stop=True)
        nc.scalar.activation(c_t[:, h * 512:(h + 1) * 512], ps_c[:],
                             AF.Tanh, bias=bct[:, 0:1])

    # --- GRU update: h' = c + u * (h - c) ---
    nc.vector.tensor_sub(d_t[:], state[0:64, :], c_t[:])
    nc.vector.tensor_mul(e_t[:], u_t[:, :], d_t[:])
    dest0, extra = lay["h_dest"], lay["h_copies"]
    nc.vector.tensor_add(dest0, c_t[:], e_t[:])
    for dst in extra:
        nc.gpsimd.tensor_copy(dst, dest0)


def _emit_support_load(nc, pools, tiles, sb, stqc, t):
    """DMA uint8 S^T for step t, convert to bf16 block-diag ST cols of sb,
    and compute S2^T cols on the PE (block-diag squared is block-diag)."""
    squ_pool, pD, pT = pools["squ"], pools["pD"], pools["pT"]
    nm_pool = pools["nm"]
    ident = tiles["ident"]

    squ = squ_pool.tile([128, PAIRS * 64], U8, tag="squ")
    nc.sync.dma_start(squ[:], stqc[t])
    squ_r = squ[:].rearrange("r (p c) -> r p c", c=64)
    sb_r = sb[:].rearrange("r (p c) -> r p c", c=256)
    # diag quadrants only; off-diag stays zero from the one-time memset
    nc.vector.tensor_copy(sb_r[0:64, :, 0:64], squ_r[0:64])
    nc.vector.tensor_copy(sb_r[64:128, :, 64:128], squ_r[64:128])

    for p in range(PAIRS):
        stb = sb[:, p * 256:p * 256 + 128]
        ps_t = pT.tile([128, 128], BF16, tag="pT")
        nc.tensor.transpose(ps_t[:, :], stb, ident[:, :])
        s_nm = nm_pool.tile([128, 128], BF16, tag="nm")
        if p % 2 == 0:
            nc.vector.tensor_copy(s_nm[:, :], ps_t[:, :])
        else:
            nc.scalar.copy(s_nm[:, :], ps_t[:, :])
        ps_s = pD.tile([128, 256], F32, tag="pD")
        nc.tensor.matmul(ps_s[:, 0:128], s_nm[:, :], stb,
                         start=True, stop=True)
        if p % 2 == 0:
            nc.scalar.copy(sb[:, p * 256 + 128:(p + 1) * 256], ps_s[:, 0:128])
        else:
            nc.vector.tensor_copy(sb[:, p * 256 + 128:(p + 1) * 256],
                                  ps_s[:, 0:128])


def _build(tin, tout):
    nc = bacc.Bacc("TRN2", target_bir_lowering=False, debug=False)

    # ---- DRAM parameters ----
    stqc = nc.declare_dram_parameter("stqc", [tin, 128, PAIRS * 64], U8,
                                     isOutput=False)
    xenc = nc.declare_dram_parameter("xenc", [tin, NT], BF16, isOutput=False)
    go = nc.declare_dram_parameter("go", [1, NT], BF16, isOutput=False)
    wgs, wcs, bgs, bcs = {}, {}, {}, {}
    for m, F in [("e0", F0), ("e1", F1), ("d0", F0), ("d1", F1)]:
        wgs[m] = nc.declare_dram_parameter(f"wg_{m}", [F, 384], BF16,
                                           isOutput=False)
        wcs[m] = nc.declare_dram_parameter(f"wc_{m}", [F, 192], BF16,
                                           isOutput=False)
        bgs[m] = nc.declare_dram_parameter(f"bg_{m}", [128, 1], F32,
                                           isOutput=False)
        bcs[m] = nc.declare_dram_parameter(f"bc_{m}", [64, 1], F32,
                                           isOutput=False)
    pw = nc.declare_dram_parameter("pw", [128, 1], BF16, isOutput=False)
    pb = nc.declare_dram_parameter("pb", [1, 1], BF16, isOutput=False)
    y = nc.declare_dram_parameter("y", [tout, NT], F32, isOutput=True)

    with tile.TileContext(nc) as tc:
        import contextlib
        with contextlib.ExitStack() as ctx:
            persist = ctx.enter_context(tc.tile_pool(name="persist", bufs=1))
            nm_pool = ctx.enter_context(tc.tile_pool(name="nm", bufs=12))
            squ_pool = ctx.enter_context(tc.tile_pool(name="squ", bufs=3))
            pT = ctx.enter_context(tc.tile_pool(name="pT", bufs=2, space="PSUM"))
            pD = ctx.enter_context(tc.tile_pool(name="pD", bufs=2, space="PSUM"))
            pG = ctx.enter_context(tc.tile_pool(name="pG", bufs=2, space="PSUM"))
            pC = ctx.enter_context(tc.tile_pool(name="pC", bufs=2, space="PSUM"))
            pools = {"pT": pT, "pD": pD, "pG": pG, "pC": pC, "nm": nm_pool,
                     "squ": squ_pool}

            ident = persist.tile([128, 128], BF16)
            make_identity(nc, ident[:])

            stss = [persist.tile([128, PAIRS * 256], BF16, name=f"stss{i}")
                    for i in range(2)]
            for s in stss:
                nc.gpsimd.memset(s[:], 0.0)

            st0 = persist.tile([F0, NT], BF16, name="st0")
            st1 = persist.tile([128, NT], BF16, name="st1")
            cnd0 = persist.tile([F0, NT], BF16, name="cnd0")
            cnd1 = persist.tile([128, NT], BF16, name="cnd1")
            cc0 = persist.tile([F0, PAIRS * 256], BF16, name="cc0")
            cc1 = persist.tile([128, PAIRS * 256], BF16, name="cc1")
            lt = {}
            for li in (0, 1):
                lt[li] = dict(
                    r=persist.tile([64, NT], BF16, name=f"r{li}"),
                    u=persist.tile([64, NT], BF16, name=f"u{li}"),
                    c=persist.tile([64, NT], BF16, name=f"c{li}"),
                    d=persist.tile([64, NT], BF16, name=f"d{li}"),
                    e=persist.tile([64, NT], BF16, name=f"e{li}"),
                )
            ones = persist.tile([1, NT], BF16, name="ones")
            nc.gpsimd.memset(ones[:], 1.0)
            ystage = persist.tile([1, NT], F32, name="ystage")

            nc.gpsimd.memset(st0[0:64, :], 0.0)
            nc.gpsimd.memset(st1[:, :], 0.0)

            wgt, wct, bgt, bct = {}, {}, {}, {}
            for m, F in [("e0", F0), ("e1", F1), ("d0", F0), ("d1", F1)]:
                wgt[m] = persist.tile([F, 384], BF16, name=f"wgt{m}")
                nc.sync.dma_start(wgt[m][:], wgs[m][:])
                wct[m] = persist.tile([F, 192], BF16, name=f"wct{m}")
                nc.sync.dma_start(wct[m][:], wcs[m][:])
                bgt[m] = persist.tile([128, 1], F32, name=f"bgt{m}")
                nc.sync.dma_start(bgt[m][:], bgs[m][:])
                bct[m] = persist.tile([64, 1], F32, name=f"bct{m}")
                nc.sync.dma_start(bct[m][:], bcs[m][:])
            pwt = persist.tile([128, 1], BF16, name="pwt")
            nc.sync.dma_start(pwt[:], pw[:])
            pbt = persist.tile([1, 1], BF16, name="pbt")
            nc.sync.dma_start(pbt[:], pb[:])

            tiles = {"ident": ident}

            # Row conventions (all h at base 0, x at the bottom):
            #   st0 [h0 (0:64), x (64:65)]    cnd0 [rh0 (0:64), x (64:65)]
            #   st1 [h1 (0:64), x=h0' (64:128)]  cnd1 [rh1 (0:64), x (64:128)]
            #   cc* rows [h-diff (0:64), x-diff (64:F)]
            # All weight matrices are row-permuted host-side to match.
            def lay0(m):
                return dict(F=F0, Dx=1, state=st0, cand=cnd0, cc=cc0,
                            wg=wgt[m], wc=wct[m], bg=bgt[m], bc=bct[m],
                            h_dest=st0[0:64, :],
                            h_copies=[st1[64:128, :], cnd1[64:128, :]],
                            **lt[0])

            def lay1(m):
                return dict(F=F1, Dx=64, state=st1, cand=cnd1, cc=cc1,
                            wg=wgt[m], wc=wct[m], bg=bgt[m], bc=bct[m],
                            h_dest=st1[0:64, :], h_copies=[], **lt[1])

            # ---------------- encoder ----------------
            for t in range(tin):
                sb = stss[t % 2]
                _emit_support_load(nc, pools, tiles, sb, stqc, t)
                nc.sync.dma_start(st0[64:65, :], xenc[t:t + 1, :])
                nc.sync.dma_start(cnd0[64:65, :], xenc[t:t + 1, :])
                _emit_cell(nc, pools, tiles, lay0("e0"), sb)
                _emit_cell(nc, pools, tiles, lay1("e1"), sb)

            # ---------------- decoder ----------------
            sb = stss[(tin - 1) % 2]
            nc.sync.dma_start(st0[64:65, :], go[:])
            nc.sync.dma_start(cnd0[64:65, :], go[:])
            for t in range(tout):
                _emit_cell(nc, pools, tiles, lay0("d0"), sb)
                _emit_cell(nc, pools, tiles, lay1("d1"), sb)
                # projection: y_t = h1' @ pw + pb   (feature-major: [1, NT])
                for h in range(2):
                    ps_p = pC.tile([64, 512], F32, tag="pC")
                    nc.tensor.matmul(ps_p[0:1, :], pwt[:, :],
                                     st1[:, h * 512:(h + 1) * 512],
                                     start=True, stop=False)
                    nc.tensor.matmul(ps_p[0:1, :], pbt[:, :],
                                     ones[:, h * 512:(h + 1) * 512],
                                     start=False, stop=True)
                    hs = slice(h * 512, (h + 1) * 512)
                    # next-step x feedback is the decoder critical path:
                    # put the two halves on different engines so they run
                    # concurrently, and demote the y staging (not on the
                    # recurrence path) behind it
                    if t < tout - 1:
                        if h == 0:
                            nc.scalar.copy(st0[64:65, hs], ps_p[0:1, :])
                        else:
                            nc.vector.tensor_copy(st0[64:65, hs],
                                                  ps_p[0:1, :])
                    if h == 0:
                        nc.vector.tensor_copy(ystage[0:1, hs], ps_p[0:1, :])
                    else:
                        nc.scalar.copy(ystage[0:1, hs], ps_p[0:1, :])
                    nc.sync.dma_start(y[t:t + 1, hs], ystage[0:1, hs])
                if t < tout - 1:
                    # off the critical path (first read is at candW time)
                    nc.gpsimd.tensor_copy(cnd0[64:65, :], st0[64:65, :])

    nc.compile()
    return nc


# ----------------------------------------------------------------------------
# cached jitted runner (shard_map over 8 cores, built once per NEFF)
# ----------------------------------------------------------------------------

def _make_runner(nc, n_cores):
    import jax
    from jax.sharding import Mesh, PartitionSpec
    from jax.experimental.shard_map import shard_map

    _b2j.install_neuronx_cc_hook()
    assert nc.dbg_addr is None, "build with debug=False"
    partition_name = (nc.partition_id_tensor.name
                      if nc.partition_id_tensor else None)

    in_names, out_names, out_avals = [], [], []
    for alloc in nc.m.functions[0].allocations:
        if not isinstance(alloc, mybir.MemoryLocationSet):
            continue
        name = alloc.memorylocations[0].name
        if alloc.kind == "ExternalInput":
            if name != partition_name:
                in_names.append(name)
        elif alloc.kind == "ExternalOutput":
            assert alloc.tensor_shape is not None and alloc.dtype is not None
            out_names.append(name)
            out_avals.append(jax.core.ShapedArray(
                tuple(alloc.tensor_shape), mybir.dt.np(alloc.dtype)))
    n_params = len(in_names)
    all_names = list(in_names) + list(out_names)
    if partition_name is not None:
        all_names.append(partition_name)
    donate = tuple(range(n_params, n_params + len(out_names)))

    def _body(*args):
        operands = list(args)
        if partition_name is not None:
            operands.append(_b2j.partition_id_tensor())
        outs = _b2j._bass_exec_p.bind(
            *operands,
            out_avals=tuple(out_avals),
            in_names=tuple(all_names),
            out_names=tuple(out_names),
            lowering_input_output_aliases=(),
            sim_require_finite=True,
            sim_require_nnan=True,
            nc=nc,
        )
        return tuple(outs)

    mesh = Mesh(np.asarray(jax.devices()[:n_cores]), ("core",))
    nin = n_params + len(out_names)
    fn = jax.jit(
        shard_map(_body, mesh=mesh,
                  in_specs=(PartitionSpec("core"),) * nin,
                  out_specs=(PartitionSpec("core"),) * len(out_names),
                  check_rep=False),
        donate_argnums=donate, keep_unused=True)
    return {"fn": fn, "in_names": in_names, "out_names": out_names,
            "out_avals": out_avals, "n_cores": n_cores}


def _get_runner(tin, tout):
    key = (tin, tout)
    if key not in _CACHE:
        nc = _build(tin, tout)
        runner = _make_runner(nc, NCORES)
        # warm call: triggers trace + XLA/neuronx compile + NEFF load now,
        # keeping subsequent calls free of one-time costs
        zeros_in = []
        for name in runner["in_names"]:
            shape, dt = _input_shape_dtype(nc, name)
            zeros_in.append(np.zeros((NCORES * shape[0], *shape[1:]), dt))
        zeros_out = [np.zeros((NCORES * a.shape[0], *a.shape[1:]), a.dtype)
                     for a in runner["out_avals"]]
        outs = runner["fn"](*zeros_in, *zeros_out)
        for o in outs:
            np.asarray(o)
        _CACHE[key] = runner
    return _CACHE[key]


def _input_shape_dtype(nc, name):
    for alloc in nc.m.functions[0].allocations:
        if not isinstance(alloc, mybir.MemoryLocationSet):
            continue
        if alloc.memorylocations[0].name == name:
            return tuple(alloc.tensor_shape), mybir.dt.np(alloc.dtype)
    raise KeyError(name)


# ----------------------------------------------------------------------------
# host side
# ----------------------------------------------------------------------------

def _prep_weights(Wg, bg, Wc, bc, F, s):
    """Split [3F, O] chebyshev-stacked weights, merge cat2 into cat0/s2 terms,
    and fold the support-quantization scale s into the diffusion blocks.

    Reference feature order within each Chebyshev block is [x (Dx), h (64)];
    on-chip tiles hold [h (0:64), x (64:F)], so every block's rows are
    permuted to [Dx:F, 0:Dx].
    """
    Dx = F - 64
    perm = list(range(Dx, F)) + list(range(Dx))
    Wg = np.asarray(Wg, np.float32)
    Wc = np.asarray(Wc, np.float32)
    w0, w1, w2 = Wg[0:F][perm], Wg[F:2 * F][perm], Wg[2 * F:3 * F][perm]
    wg = np.concatenate([w0 - w2, s * w1, (2.0 * s * s) * w2], axis=1)
    c0, c1, c2 = Wc[0:F][perm], Wc[F:2 * F][perm], Wc[2 * F:3 * F][perm]
    wc = np.concatenate([c0 - c2, s * c1, (2.0 * s * s) * c2], axis=1)
    return (wg.astype(ml_dtypes.bfloat16), wc.astype(ml_dtypes.bfloat16),
            np.asarray(bg, np.float32).reshape(-1, 1),
            np.asarray(bc, np.float32).reshape(-1, 1))


def kernel(encoder_inputs, decoder_inputs, supports,
           enc0_Wg, enc0_bg, enc0_Wc, enc0_bc,
           enc1_Wg, enc1_bg, enc1_Wc, enc1_bc,
           dec0_Wg, dec0_bg, dec0_Wc, dec0_bc,
           dec1_Wg, dec1_bg, dec1_Wc, dec1_bc,
           proj_W, proj_b):
    encoder_inputs = np.asarray(encoder_inputs, np.float32)
    decoder_inputs = np.asarray(decoder_inputs, np.float32)
    supports = np.asarray(supports, np.float32)
    Bv, tin, Nv, _ = encoder_inputs.shape
    tout = decoder_inputs.shape[1]

    runner = _get_runner(tin, tout)

    # ---- supports: uint8-quantize S^T into [ncores*tin, 128, PAIRS*64] ----
    # G[core*tin + t, r, p*64 + c]:
    #   r <  64: ST[core*BL + 2p,     t, r,      c]   (even sample of pair)
    #   r >= 64: ST[core*BL + 2p + 1, t, r - 64, c]   (odd sample)
    smax = float(supports.max())
    s = smax / 255.0 if smax > 0 else 1.0
    inv_s = np.float32(1.0 / s)
    Sv = supports.reshape(NCORES, BL, tin, Nv, Nv)
    G = np.empty((NCORES, tin, 128, PAIRS, 64), np.uint8)
    # ST[b,t,r,c] = S[b,t,c,r]; (core,pair,t,i,j) -> (core,t,j,pair,i)
    G[:, :, 0:64] = Sv[:, 0::2].transpose(0, 2, 4, 1, 3) * inv_s + 0.5
    G[:, :, 64:128] = Sv[:, 1::2].transpose(0, 2, 4, 1, 3) * inv_s + 0.5
    G = G.reshape(NCORES * tin, 128, PAIRS * 64)

    xe = np.ascontiguousarray(
        encoder_inputs[:, :, :, 0].reshape(NCORES, BL, tin, Nv)
        .transpose(0, 2, 1, 3)).reshape(NCORES * tin, NT).astype(
            ml_dtypes.bfloat16)
    go_h = decoder_inputs[:, 0, :, 0].reshape(NCORES, NT).astype(
        ml_dtypes.bfloat16)

    wg_e0, wc_e0, bg_e0, bc_e0 = _prep_weights(enc0_Wg, enc0_bg, enc0_Wc,
                                               enc0_bc, F0, s)
    wg_e1, wc_e1, bg_e1, bc_e1 = _prep_weights(enc1_Wg, enc1_bg, enc1_Wc,
                                               enc1_bc, F1, s)
    wg_d0, wc_d0, bg_d0, bc_d0 = _prep_weights(dec0_Wg, dec0_bg, dec0_Wc,
                                               dec0_bc, F0, s)
    wg_d1, wc_d1, bg_d1, bc_d1 = _prep_weights(dec1_Wg, dec1_bg, dec1_Wc,
                                               dec1_bc, F1, s)
    pw_h = np.zeros((128, 1), np.float32)
    pw_h[0:64] = np.asarray(proj_W, np.float32).reshape(64, 1)
    pw_h = pw_h.astype(ml_dtypes.bfloat16)
    pb_h = np.asarray(proj_b, np.float32).reshape(1, 1).astype(
        ml_dtypes.bfloat16)

    def rep(a):
        return np.concatenate([a] * NCORES, axis=0)

    per_input = {
        "stqc": G, "xenc": xe, "go": go_h,
        "wg_e0": rep(wg_e0), "wc_e0": rep(wc_e0),
        "bg_e0": rep(bg_e0), "bc_e0": rep(bc_e0),
        "wg_e1": rep(wg_e1), "wc_e1": rep(wc_e1),
        "bg_e1": rep(bg_e1), "bc_e1": rep(bc_e1),
        "wg_d0": rep(wg_d0), "wc_d0": rep(wc_d0),
        "bg_d0": rep(bg_d0), "bc_d0": rep(bc_d0),
        "wg_d1": rep(wg_d1), "wc_d1": rep(wc_d1),
        "bg_d1": rep(bg_d1), "bc_d1": rep(bc_d1),
        "pw": rep(pw_h), "pb": rep(pb_h),
    }

    global last_exec_wall_ns
    import time as _time
    _t0 = _time.time()
    args_in = [per_input[name] for name in runner["in_names"]]
    args_out = [np.zeros((NCORES * a.shape[0], *a.shape[1:]), a.dtype)
                for a in runner["out_avals"]]
    out_arrs = runner["fn"](*args_in, *args_out)
    iy = runner["out_names"].index("y")
    yg = np.asarray(out_arrs[iy])            # (NCORES*tout, NT) f32
    last_exec_wall_ns = int((_time.time() - _t0) * 1e9)

    out = np.empty((Bv, tout, Nv, 1), np.float32)
    yg = yg.reshape(NCORES, tout, BL, Nv)
    for c in range(NCORES):
        out[c * BL:(c + 1) * BL, :, :, 0] = np.transpose(yg[c], (1, 0, 2))
    return out


# revision 10
# speedup vs baseline: 25.1344x; 3.0521x over previous
"""DCRNN (2-layer DCGRU encoder/decoder, K=2 Chebyshev) Trainium2 kernel.

Sharding: pure data-parallel over batch B=128 -> 16 samples per core x 8 cores.

Layouts (per core, BL=16 samples, N=64 nodes, NT=BL*N=1024):
  feature-major state tiles: [feat_partition, 64*b + n]
  samples paired (2 per 128-partition group) for block-diagonal support matmuls.

Host->device traffic is the bottleneck (axon tunnel ~100MB/s), so supports are
shipped once per step as uint8-quantized S^T only ([tin,128,512] contiguous);
the device converts u8->bf16 into the block-diag pair layout and computes
S2^T = S^T @ S^T itself (block-diag squared stays block-diag).  The
quantization scale s is folded into the gate/candidate weights host-side
(diffusion terms are linear in S): blocks [W0 - W2, s*W1, 2*s^2*W2].

Per DCGRU cell (layer l, feature dim F = Dx + 64):
  gate = sigmoid(cat0 @ Wg0' + (Si@cat0) @ (s Wg1) + (Si2@cat0) @ (2 s^2 Wg2) + bg)
  computed feature-major via: per-pair PE transpose of cat0 (fm->nm), one
  matmul per pair against [SiT|Si2T] block-diag tiles (fm diffusion outputs),
  then weight matmuls with W stationary streaming all 16 samples.

Execution path: the jitted shard_map(bass_exec) callable is cached per build —
re-creating it per call (as run_bass_kernel_spmd does under axon) re-lowers
and re-loads the NEFF, costing seconds per call.
"""

import numpy as np
import ml_dtypes

import concourse.bass as bass
import concourse.mybir as mybir
import concourse.tile as tile
from concourse import bacc
from concourse import bass2jax as _b2j
from concourse.masks import make_identity

F32 = mybir.dt.float32
BF16 = mybir.dt.bfloat16
U8 = mybir.dt.uint8
AF = mybir.ActivationFunctionType

B, TIN, TOUT, N, H = 128, 64, 32, 64, 64
NCORES = 8
BL = B // NCORES          # 16 samples per core
PAIRS = BL // 2           # 8
NT = BL * N               # 1024 node-columns per core
F0, F1 = 1 + H, H + H     # 65, 128

_CACHE = {}
last_exec_wall_ns = None  # wall time of the device dispatch in the last call


# ----------------------------------------------------------------------------
# device kernel builder
# ----------------------------------------------------------------------------

def _emit_cell(nc, pools, tiles, lay, sbuf_sts, dbg=""):
    """Emit one DCGRU cell. lay: dict with F, Dx, state, cand, cc, wg, wc,
    bg, bc, h_dests (list of (tile, row0) to write h' into).

    Precision: matmul inputs are bf16, but the recurrent state lives in an
    f32 master tile (hf); gates r/u/c are kept f32 and the GRU update runs
    in f32 so state rounding does not compound across the 96 steps.  bf16
    enters only as a fresh rounding of matmul inputs each step."""
    F, Dx = lay["F"], lay["Dx"]
    state, cand, cc = lay["state"], lay["cand"], lay["cc"]
    wg, wc, bgt, bct = lay["wg"], lay["wc"], lay["bg"], lay["bc"]
    ident = tiles["ident"]
    r_t, u_t = lay["r"], lay["u"]
    c_t, d_t, e_t = lay["c"], lay["d"], lay["e"]
    hf = lay["hf"]
    pT, pD, pG, pC = pools["pT"], pools["pD"], pools["pG"], pools["pC"]
    nm_pool = pools["nm"]

    # --- gate path: per-pair transpose + diffusion ---
    for p in range(PAIRS):
        ps_t1 = pT.tile([128, 128], BF16, tag="pT")
        nc.tensor.transpose(ps_t1[:, :F], state[:, p * 128:(p + 1) * 128],
                            ident[:F, :F])
        cat0nm = nm_pool.tile([128, 128], BF16, tag="nm")
        nc.vector.tensor_copy(cat0nm[:, :F], ps_t1[:, :F])
        ps_d1 = pD.tile([128, 256], F32, tag="pD")
        nc.tensor.matmul(ps_d1[:F, :], cat0nm[:, :F],
                         sbuf_sts[:, p * 256:(p + 1) * 256],
                         start=True, stop=True)
        # alternate copy engine: ACT copies are ~2x slower than DVE, so
        # split the 8 per-pair copies between the two engines
        if p % 2 == 0:
            nc.vector.tensor_copy(cc[:F, p * 256:(p + 1) * 256], ps_d1[:F, :])
        else:
            nc.scalar.copy(cc[:F, p * 256:(p + 1) * 256], ps_d1[:F, :])

    # --- gate weight matmuls (W stationary, all samples streamed) ---
    cc_r = cc[:].rearrange("f (p c) -> f p c", c=256)
    for h in range(2):
        ps_g = pG.tile([128, 512], F32, tag="pG")
        nc.tensor.matmul(ps_g[:], wg[:, 0:128], state[:, h * 512:(h + 1) * 512],
                         start=True, stop=False)
        nc.tensor.matmul(ps_g[:], wg[:, 128:256],
                         cc_r[:F, 4 * h:4 * h + 4, 0:128],
                         start=False, stop=False)
        nc.tensor.matmul(ps_g[:], wg[:, 256:384],
                         cc_r[:F, 4 * h:4 * h + 4, 128:256],
                         start=False, stop=True)
        nc.scalar.activation(r_t[:, h * 512:(h + 1) * 512], ps_g[0:64, :],
                             AF.Sigmoid, bias=bgt[0:64, 0:1])
        nc.scalar.activation(u_t[:, h * 512:(h + 1) * 512], ps_g[64:128, :],
                             AF.Sigmoid, bias=bgt[64:128, 0:1])

    # --- candidate path ---
    # rh = r * h  (f32 inputs, one fresh bf16 rounding) into cand rows [0, 64)
    nc.vector.tensor_mul(cand[0:64, :], r_t[:, :], hf[:, :])
    for p in range(PAIRS):
        ps_t2 = pT.tile([128, 128], BF16, tag="pT")
        nc.tensor.transpose(ps_t2[:, :64], cand[0:64, p * 128:(p + 1) * 128],
                            ident[0:64, 0:64])
        rhnm = nm_pool.tile([128, 128], BF16, tag="nm")
        if p % 2 == 0:
            nc.vector.tensor_copy(rhnm[:, :64], ps_t2[:, :64])
        else:
            nc.scalar.copy(rhnm[:, :64], ps_t2[:, :64])
        ps_d2 = pD.tile([128, 256], F32, tag="pD")
        nc.tensor.matmul(ps_d2[:64, :], rhnm[:, :64],
                         sbuf_sts[:, p * 256:(p + 1) * 256],
                         start=True, stop=True)
        if p % 2 == 0:
            nc.vector.tensor_copy(cc[0:64, p * 256:(p + 1) * 256],
                                  ps_d2[:64, :])
        else:
            nc.scalar.copy(cc[0:64, p * 256:(p + 1) * 256], ps_d2[:64, :])

    for h in range(2):
        ps_c = pC.tile([64, 512], F32, tag="pC")
        nc.tensor.matmul(ps_c[:], wc[:, 0:64], cand[:, h * 512:(h + 1) * 512],
                         start=True, stop=False)
        nc.tensor.matmul(ps_c[:], wc[:, 64:128],
                         cc_r[:F, 4 * h:4 * h + 4, 0:128],
                         start=False, stop=False)
        nc.tensor.matmul(ps_c[:], wc[:, 128:192],
                         cc_r[:F, 4 * h:4 * h + 4, 128:256],
                         start=False, stop=True)
        nc.scalar.activation(c_t[:, h * 512:(h + 1) * 512], ps_c[:],
                             AF.Tanh, bias=bct[:, 0:1])

    # --- GRU update: h' = c + u * (h - c), all f32 on the master state ---
    nc.vector.tensor_sub(d_t[:], hf[:], c_t[:])
    nc.vector.tensor_mul(e_t[:], u_t[:, :], d_t[:])
    dest0, extra = lay["h_dest"], lay["h_copies"]
    nc.vector.tensor_add(hf[:], c_t[:], e_t[:])
    nc.vector.tensor_copy(dest0, hf[:])
    for dst in extra:
        nc.gpsimd.tensor_copy(dst, dest0)


def _emit_support_load(nc, pools, tiles, sb, stqc, t):
    """DMA uint8 S^T for step t, convert to bf16 block-diag ST cols of sb,
    and compute S2^T cols on the PE (block-diag squared is block-diag)."""
    squ_pool, pD, pT = pools["squ"], pools["pD"], pools["pT"]
    nm_pool = pools["nm"]
    ident = tiles["ident"]

    squ = squ_pool.tile([128, PAIRS * 64], U8, tag="squ")
    nc.sync.dma_start(squ[:], stqc[t])
    squ_r = squ[:].rearrange("r (p c) -> r p c", c=64)
    sb_r = sb[:].rearrange("r (p c) -> r p c", c=256)
    # diag quadrants only; off-diag stays zero from the one-time memset
    nc.vector.tensor_copy(sb_r[0:64, :, 0:64], squ_r[0:64])
    nc.vector.tensor_copy(sb_r[64:128, :, 64:128], squ_r[64:128])

    for p in range(PAIRS):
        stb = sb[:, p * 256:p * 256 + 128]
        ps_t = pT.tile([128, 128], BF16, tag="pT")
        nc.tensor.transpose(ps_t[:, :], stb, ident[:, :])
        s_nm = nm_pool.tile([128, 128], BF16, tag="nm")
        if p % 2 == 0:
            nc.vector.tensor_copy(s_nm[:, :], ps_t[:, :])
        else:
            nc.scalar.copy(s_nm[:, :], ps_t[:, :])
        ps_s = pD.tile([128, 256], F32, tag="pD")
        nc.tensor.matmul(ps_s[:, 0:128], s_nm[:, :], stb,
                         start=True, stop=True)
        if p % 2 == 0:
            nc.scalar.copy(sb[:, p * 256 + 128:(p + 1) * 256], ps_s[:, 0:128])
        else:
            nc.vector.tensor_copy(sb[:, p * 256 + 128:(p + 1) * 256],
                                  ps_s[:, 0:128])


def _build(tin, tout):
    nc = bacc.Bacc("TRN2", target_bir_lowering=False, debug=False)

    # ---- DRAM parameters ----
    stqc = nc.declare_dram_parameter("stqc", [tin, 128, PAIRS * 64], U8,
                                     isOutput=False)
    xenc = nc.declare_dram_parameter("xenc", [tin, NT], BF16, isOutput=False)
    go = nc.declare_dram_parameter("go", [1, NT], BF16, isOutput=False)
    wgs, wcs, bgs, bcs = {}, {}, {}, {}
    for m, F in [("e0", F0), ("e1", F1), ("d0", F0), ("d1", F1)]:
        wgs[m] = nc.declare_dram_parameter(f"wg_{m}", [F, 384], BF16,
                                           isOutput=False)
        wcs[m] = nc.declare_dram_parameter(f"wc_{m}", [F, 192], BF16,
                                           isOutput=False)
        bgs[m] = nc.declare_dram_parameter(f"bg_{m}", [128, 1], F32,
                                           isOutput=False)
        bcs[m] = nc.declare_dram_parameter(f"bc_{m}", [64, 1], F32,
                                           isOutput=False)
    pw = nc.declare_dram_parameter("pw", [128, 1], BF16, isOutput=False)
    pb = nc.declare_dram_parameter("pb", [1, 1], BF16, isOutput=False)
    y = nc.declare_dram_parameter("y", [tout, NT], F32, isOutput=True)

    with tile.TileContext(nc) as tc:
        import contextlib
        with contextlib.ExitStack() as ctx:
            persist = ctx.enter_context(tc.tile_pool(name="persist", bufs=1))
            nm_pool = ctx.enter_context(tc.tile_pool(name="nm", bufs=12))
            squ_pool = ctx.enter_context(tc.tile_pool(name="squ", bufs=3))
            pT = ctx.enter_context(tc.tile_pool(name="pT", bufs=2, space="PSUM"))
            pD = ctx.enter_context(tc.tile_pool(name="pD", bufs=2, space="PSUM"))
            pG = ctx.enter_context(tc.tile_pool(name="pG", bufs=2, space="PSUM"))
            pC = ctx.enter_context(tc.tile_pool(name="pC", bufs=2, space="PSUM"))
            pools = {"pT": pT, "pD": pD, "pG": pG, "pC": pC, "nm": nm_pool,
                     "squ": squ_pool}

            ident = persist.tile([128, 128], BF16)
            make_identity(nc, ident[:])

            stss = [persist.tile([128, PAIRS * 256], BF16, name=f"stss{i}")
                    for i in range(2)]
            for s in stss:
                nc.gpsimd.memset(s[:], 0.0)

            st0 = persist.tile([F0, NT], BF16, name="st0")
            st1 = persist.tile([128, NT], BF16, name="st1")
            cnd0 = persist.tile([F0, NT], BF16, name="cnd0")
            cnd1 = persist.tile([128, NT], BF16, name="cnd1")
            cc0 = persist.tile([F0, PAIRS * 256], BF16, name="cc0")
            cc1 = persist.tile([128, PAIRS * 256], BF16, name="cc1")
            lt = {}
            for li in (0, 1):
                lt[li] = dict(
                    r=persist.tile([64, NT], F32, name=f"r{li}"),
                    u=persist.tile([64, NT], F32, name=f"u{li}"),
                    c=persist.tile([64, NT], F32, name=f"c{li}"),
                    d=persist.tile([64, NT], F32, name=f"d{li}"),
                    e=persist.tile([64, NT], F32, name=f"e{li}"),
                    hf=persist.tile([64, NT], F32, name=f"hf{li}"),
                )
            ones = persist.tile([1, NT], BF16, name="ones")
            nc.gpsimd.memset(ones[:], 1.0)
            ystage = persist.tile([1, NT], F32, name="ystage")

            nc.gpsimd.memset(st0[0:64, :], 0.0)
            nc.gpsimd.memset(st1[:, :], 0.0)
            nc.gpsimd.memset(lt[0]["hf"][:], 0.0)
            nc.gpsimd.memset(lt[1]["hf"][:], 0.0)

            wgt, wct, bgt, bct = {}, {}, {}, {}
            for m, F in [("e0", F0), ("e1", F1), ("d0", F0), ("d1", F1)]:
                wgt[m] = persist.tile([F, 384], BF16, name=f"wgt{m}")
                nc.sync.dma_start(wgt[m][:], wgs[m][:])
                wct[m] = persist.tile([F, 192], BF16, name=f"wct{m}")
                nc.sync.dma_start(wct[m][:], wcs[m][:])
                bgt[m] = persist.tile([128, 1], F32, name=f"bgt{m}")
                nc.sync.dma_start(bgt[m][:], bgs[m][:])
                bct[m] = persist.tile([64, 1], F32, name=f"bct{m}")
                nc.sync.dma_start(bct[m][:], bcs[m][:])
            pwt = persist.tile([128, 1], BF16, name="pwt")
            nc.sync.dma_start(pwt[:], pw[:])
            pbt = persist.tile([1, 1], BF16, name="pbt")
            nc.sync.dma_start(pbt[:], pb[:])

            tiles = {"ident": ident}

            # Row conventions (all h at base 0, x at the bottom):
            #   st0 [h0 (0:64), x (64:65)]    cnd0 [rh0 (0:64), x (64:65)]
            #   st1 [h1 (0:64), x=h0' (64:128)]  cnd1 [rh1 (0:64), x (64:128)]
            #   cc* rows [h-diff (0:64), x-diff (64:F)]
            # All weight matrices are row-permuted host-side to match.
            def lay0(m):
                return dict(F=F0, Dx=1, state=st0, cand=cnd0, cc=cc0,
                            wg=wgt[m], wc=wct[m], bg=bgt[m], bc=bct[m],
                            h_dest=st0[0:64, :],
                            h_copies=[st1[64:128, :], cnd1[64:128, :]],
                            **lt[0])

            def lay1(m):
                return dict(F=F1, Dx=64, state=st1, cand=cnd1, cc=cc1,
                            wg=wgt[m], wc=wct[m], bg=bgt[m], bc=bct[m],
                            h_dest=st1[0:64, :], h_copies=[], **lt[1])

            # ---------------- encoder ----------------
            for t in range(tin):
                sb = stss[t % 2]
                _emit_support_load(nc, pools, tiles, sb, stqc, t)
                nc.sync.dma_start(st0[64:65, :], xenc[t:t + 1, :])
                nc.sync.dma_start(cnd0[64:65, :], xenc[t:t + 1, :])
                _emit_cell(nc, pools, tiles, lay0("e0"), sb)
                _emit_cell(nc, pools, tiles, lay1("e1"), sb)

            # ---------------- decoder ----------------
            sb = stss[(tin - 1) % 2]
            nc.sync.dma_start(st0[64:65, :], go[:])
            nc.sync.dma_start(cnd0[64:65, :], go[:])
            for t in range(tout):
                _emit_cell(nc, pools, tiles, lay0("d0"), sb)
                _emit_cell(nc, pools, tiles, lay1("d1"), sb)
                # projection: y_t = h1' @ pw + pb   (feature-major: [1, NT])
                for h in range(2):
                    ps_p = pC.tile([64, 512], F32, tag="pC")
                    nc.tensor.matmul(ps_p[0:1, :], pwt[:, :],
                                     st1[:, h * 512:(h + 1) * 512],
                                     start=True, stop=False)
                    nc.tensor.matmul(ps_p[0:1, :], pbt[:, :],
                                     ones[:, h * 512:(h + 1) * 512],
                                     start=False, stop=True)
                    hs = slice(h * 512, (h + 1) * 512)
                    # next-step x feedback is the decoder critical path:
                    # put the two halves on different engines so they run
                    # concurrently, and demote the y staging (not on the
                    # recurrence path) behind it
                    if t < tout - 1:
                        if h == 0:
                            nc.scalar.copy(st0[64:65, hs], ps_p[0:1, :])
                        else:
                            nc.vector.tensor_copy(st0[64:65, hs],
                                                  ps_p[0:1, :])
                    if h == 0:
                        nc.vector.tensor_copy(ystage[0:1, hs], ps_p[0:1, :])
                    else:
                        nc.scalar.copy(ystage[0:1, hs], ps_p[0:1, :])
                    nc.sync.dma_start(y[t:t + 1, hs], ystage[0:1, hs])
                if t < tout - 1:
                    # off the critical path (first read is at candW time)
                    nc.gpsimd.tensor_copy(cnd0[64:65, :], st0[64:65, :])

    nc.compile()
    return nc


# ----------------------------------------------------------------------------
# cached jitted runner (shard_map over 8 cores, built once per NEFF)
# ----------------------------------------------------------------------------

def _make_runner(nc, n_cores):
    import jax
    from jax.sharding import Mesh, PartitionSpec
    from jax.experimental.shard_map import shard_map

    _b2j.install_neuronx_cc_hook()
    assert nc.dbg_addr is None, "build with debug=False"
    partition_name = (nc.partition_id_tensor.name
                      if nc.partition_id_tensor else None)

    in_names, out_names, out_avals = [], [], []
    for alloc in nc.m.functions[0].allocations:
        if not isinstance(alloc, mybir.MemoryLocationSet):
            continue
        name = alloc.memorylocations[0].name
        if alloc.kind == "ExternalInput":
            if name != partition_name:
                in_names.append(name)
        elif alloc.kind == "ExternalOutput":
            assert alloc.tensor_shape is not None and alloc.dtype is not None
            out_names.append(name)
            out_avals.append(jax.core.ShapedArray(
                tuple(alloc.tensor_shape), mybir.dt.np(alloc.dtype)))
    n_params = len(in_names)
    all_names = list(in_names) + list(out_names)
    if partition_name is not None:
        all_names.append(partition_name)
    donate = tuple(range(n_params, n_params + len(out_names)))

    def _body(*args):
        operands = list(args)
        if partition_name is not None:
            operands.append(_b2j.partition_id_tensor())
        outs = _b2j._bass_exec_p.bind(
            *operands,
            out_avals=tuple(out_avals),
            in_names=tuple(all_names),
            out_names=tuple(out_names),
            lowering_input_output_aliases=(),
            sim_require_finite=True,
            sim_require_nnan=True,
            nc=nc,
        )
        return tuple(outs)

    mesh = Mesh(np.asarray(jax.devices()[:n_cores]), ("core",))
    nin = n_params + len(out_names)
    fn = jax.jit(
        shard_map(_body, mesh=mesh,
                  in_specs=(PartitionSpec("core"),) * nin,
                  out_specs=(PartitionSpec("core"),) * len(out_names),
                  check_rep=False),
        donate_argnums=donate, keep_unused=True)
    return {"fn": fn, "in_names": in_names, "out_names": out_names,
            "out_avals": out_avals, "n_cores": n_cores}


def _get_runner(tin, tout):
    key = (tin, tout)
    if key not in _CACHE:
        nc = _build(tin, tout)
        runner = _make_runner(nc, NCORES)
        # warm call: triggers trace + XLA/neuronx compile + NEFF load now,
        # keeping subsequent calls free of one-time costs
        zeros_in = []
        for name in runner["in_names"]:
            shape, dt = _input_shape_dtype(nc, name)
            zeros_in.append(np.zeros((NCORES * shape[0], *shape[1:]), dt))
        zeros_out = [np.zeros((NCORES * a.shape[0], *a.shape[1:]), a.dtype)
                     for a in runner["out_avals"]]
        outs = runner["fn"](*zeros_in, *zeros_out)
        for o in outs:
            np.asarray(o)
        _CACHE[key] = runner
    return _CACHE[key]


def _input_shape_dtype(nc, name):
    for alloc in nc.m.functions[0].allocations:
        if not isinstance(alloc, mybir.MemoryLocationSet):
            continue
        if alloc.memorylocations[0].name == name:
            return tuple(alloc.tensor_shape), mybir.dt.np(alloc.dtype)
    raise KeyError(name)


# ----------------------------------------------------------------------------
# host side
# ----------------------------------------------------------------------------

def _prep_weights(Wg, bg, Wc, bc, F, s):
    """Split [3F, O] chebyshev-stacked weights, merge cat2 into cat0/s2 terms,
    and fold the support-quantization scale s into the diffusion blocks.

    Reference feature order within each Chebyshev block is [x (Dx), h (64)];
    on-chip tiles hold [h (0:64), x (64:F)], so every block's rows are
    permuted to [Dx:F, 0:Dx].
    """
    Dx = F - 64
    perm = list(range(Dx, F)) + list(range(Dx))
    Wg = np.asarray(Wg, np.float32)
    Wc = np.asarray(Wc, np.float32)
    w0, w1, w2 = Wg[0:F][perm], Wg[F:2 * F][perm], Wg[2 * F:3 * F][perm]
    wg = np.concatenate([w0 - w2, s * w1, (2.0 * s * s) * w2], axis=1)
    c0, c1, c2 = Wc[0:F][perm], Wc[F:2 * F][perm], Wc[2 * F:3 * F][perm]
    wc = np.concatenate([c0 - c2, s * c1, (2.0 * s * s) * c2], axis=1)
    return (wg.astype(ml_dtypes.bfloat16), wc.astype(ml_dtypes.bfloat16),
            np.asarray(bg, np.float32).reshape(-1, 1),
            np.asarray(bc, np.float32).reshape(-1, 1))


def kernel(encoder_inputs, decoder_inputs, supports,
           enc0_Wg, enc0_bg, enc0_Wc, enc0_bc,
           enc1_Wg, enc1_bg, enc1_Wc, enc1_bc,
           dec0_Wg, dec0_bg, dec0_Wc, dec0_bc,
           dec1_Wg, dec1_bg, dec1_Wc, dec1_bc,
           proj_W, proj_b):
    encoder_inputs = np.asarray(encoder_inputs, np.float32)
    decoder_inputs = np.asarray(decoder_inputs, np.float32)
    supports = np.asarray(supports, np.float32)
    Bv, tin, Nv, _ = encoder_inputs.shape
    tout = decoder_inputs.shape[1]

    runner = _get_runner(tin, tout)

    # ---- supports: uint8-quantize S^T into [ncores*tin, 128, PAIRS*64] ----
    # G[core*tin + t, r, p*64 + c]:
    #   r <  64: ST[core*BL + 2p,     t, r,      c]   (even sample of pair)
    #   r >= 64: ST[core*BL + 2p + 1, t, r - 64, c]   (odd sample)
    smax = float(supports.max())
    s = smax / 255.0 if smax > 0 else 1.0
    inv_s = np.float32(1.0 / s)
    Sv = supports.reshape(NCORES, BL, tin, Nv, Nv)
    G = np.empty((NCORES, tin, 128, PAIRS, 64), np.uint8)
    # ST[b,t,r,c] = S[b,t,c,r]; (core,pair,t,i,j) -> (core,t,j,pair,i)
    G[:, :, 0:64] = Sv[:, 0::2].transpose(0, 2, 4, 1, 3) * inv_s + 0.5
    G[:, :, 64:128] = Sv[:, 1::2].transpose(0, 2, 4, 1, 3) * inv_s + 0.5
    G = G.reshape(NCORES * tin, 128, PAIRS * 64)

    xe = np.ascontiguousarray(
        encoder_inputs[:, :, :, 0].reshape(NCORES, BL, tin, Nv)
        .transpose(0, 2, 1, 3)).reshape(NCORES * tin, NT).astype(
            ml_dtypes.bfloat16)
    go_h = decoder_inputs[:, 0, :, 0].reshape(NCORES, NT).astype(
        ml_dtypes.bfloat16)

    wg_e0, wc_e0, bg_e0, bc_e0 = _prep_weights(enc0_Wg, enc0_bg, enc0_Wc,
                                               enc0_bc, F0, s)
    wg_e1, wc_e1, bg_e1, bc_e1 = _prep_weights(enc1_Wg, enc1_bg, enc1_Wc,
                                               enc1_bc, F1, s)
    wg_d0, wc_d0, bg_d0, bc_d0 = _prep_weights(dec0_Wg, dec0_bg, dec0_Wc,
                                               dec0_bc, F0, s)
    wg_d1, wc_d1, bg_d1, bc_d1 = _prep_weights(dec1_Wg, dec1_bg, dec1_Wc,
                                               dec1_bc, F1, s)
    pw_h = np.zeros((128, 1), np.float32)
    pw_h[0:64] = np.asarray(proj_W, np.float32).reshape(64, 1)
    pw_h = pw_h.astype(ml_dtypes.bfloat16)
    pb_h = np.asarray(proj_b, np.float32).reshape(1, 1).astype(
        ml_dtypes.bfloat16)

    def rep(a):
        return np.concatenate([a] * NCORES, axis=0)

    per_input = {
        "stqc": G, "xenc": xe, "go": go_h,
        "wg_e0": rep(wg_e0), "wc_e0": rep(wc_e0),
        "bg_e0": rep(bg_e0), "bc_e0": rep(bc_e0),
        "wg_e1": rep(wg_e1), "wc_e1": rep(wc_e1),
        "bg_e1": rep(bg_e1), "bc_e1": rep(bc_e1),
        "wg_d0": rep(wg_d0), "wc_d0": rep(wc_d0),
        "bg_d0": rep(bg_d0), "bc_d0": rep(bc_d0),
        "wg_d1": rep(wg_d1), "wc_d1": rep(wc_d1),
        "bg_d1": rep(bg_d1), "bc_d1": rep(bc_d1),
        "pw": rep(pw_h), "pb": rep(pb_h),
    }

    global last_exec_wall_ns
    import time as _time
    _t0 = _time.time()
    args_in = [per_input[name] for name in runner["in_names"]]
    args_out = [np.zeros((NCORES * a.shape[0], *a.shape[1:]), a.dtype)
                for a in runner["out_avals"]]
    out_arrs = runner["fn"](*args_in, *args_out)
    iy = runner["out_names"].index("y")
    yg = np.asarray(out_arrs[iy])            # (NCORES*tout, NT) f32
    last_exec_wall_ns = int((_time.time() - _t0) * 1e9)

    out = np.empty((Bv, tout, Nv, 1), np.float32)
    yg = yg.reshape(NCORES, tout, BL, Nv)
    for c in range(NCORES):
        out[c * BL:(c + 1) * BL, :, :, 0] = np.transpose(yg[c], (1, 0, 2))
    return out
